# revision 30
# baseline (speedup 1.0000x reference)
"""DiffLogic network TRN2 kernel: 3 logic layers [B=256, W=64000] + GroupSum.

Sharding: pure data-parallel over batch across 8 cores (B=32/core).
Key structure (per core):
- Dead-neuron pruning: L2 neurons never gathered by L3, and L1 neurons
  never gathered by the pruned L2, are dropped entirely (~13%/18%).
- h stored in DRAM as 64B rows at 256B stride; gathers via SWDGE
  dma_gather with 64B descriptors (7ns floor) and signed int16 idxs
  (per-chunk rebase, negative idxs reach backward from the AP base).
- L2/L3 gather a+b operands with ONE merged dma_gather per chunk
  (idx stream = [a-idxs, b-idxs], shared rebase) so Pool desc-gen
  keeps up with the DMA engines.
- Layers are pipelined: consumers sorted by max(source row) so each
  gather's src window only covers already-written h rows.
- L1's a-operand is an affine broadcast from a compact x table
  (consumers placed in their a-source's grid cell; spill tail gathers).
- Gate = (c1+c3*b)*a + (c0+c2*b) on DVE (6 ops, 2x perf mode) for
  L1/L2. L3 skips the gate: out_k = sum(c0) + sum(c1 a) + sum(c2 b)
  + sum(c3 ab) via per-slot coefficient-column PE matmuls (rhs = the
  gathered a, b tiles and one DVE ab product); slots are group-pure
  (host bucketing in max-order) and each group accumulates in its own
  2KB PSUM bank (start=True zeroes the whole bank region).
- coefs = softmax(w)@G via ACT exp (fp8 weights) + PE matmuls.
"""
import numpy as np
import ml_dtypes

import concourse.bass as bass
import concourse.tile as tile
import concourse.bacc as bacc
import concourse.mybir as mybir
from concourse import ap_utils
from concourse.bass_utils import run_bass_kernel_spmd
from concourse.library_config import mlp

W = 64000
BATCH = 256
NCORES = 8
BC = BATCH // NCORES        # 32 batch rows per core
IN_DIM = 1024
K = 10
TAU = 30.0
NSLOT = W // 128            # 500
E = 128                     # elements per h row (256B stride); [:32] real
CHUNK_SLOTS = 64            # L1 b-gather chunk (neurons = 64*128 = 8192)
MCHUNK = 60                 # L2 merged a+b chunk: 2*60*128 idxs -> 961 ring descs
GPN = 8192                  # idxs per dma_gather instruction (needs single_packet=False)
H_BASE = 32000              # gather base row (signed int16 rebase)

GATE_COEF = np.array([
    [0., 0., 0., 0.], [0., 0., 0., 1.], [0., 1., 0., -1.], [0., 1., 0., 0.],
    [0., 0., 1., -1.], [0., 0., 1., 0.], [0., 1., 1., -2.], [0., 1., 1., -1.],
    [1., -1., -1., 1.], [1., -1., -1., 2.], [1., 0., -1., 0.], [1., 0., -1., 1.],
    [1., -1., 0., 0.], [1., -1., 0., 1.], [1., 0., 0., -1.], [1., 0., 0., 0.],
], dtype=np.float32)  # [16, 4]

BF16 = mybir.dt.bfloat16
F32 = mybir.dt.float32
I16 = mybir.dt.int16

_NC_CACHE = {}


def dma_gather_small(gp, out_ap, in_ap, idxs_ap, num_idxs, elem_size, elem_step):
    """dma_gather without the 256B elem-size assert (non-transpose, DRAM src).
    The 256B restriction only applies to transpose mode in the ucode; the
    source row stride (elem_step) must still be a multiple of 256B."""
    assert idxs_ap.dtype == mybir.dt.int16
    assert in_ap.dtype == out_ap.dtype
    assert ap_utils.ap_is_contiguous(out_ap.ap[1:])
    assert ap_utils.ap_is_contiguous(idxs_ap.ap[1:])
    assert in_ap.ap[-1][1] == out_ap.ap[-1][1] == elem_size
    assert in_ap.ap[0][0] == elem_step
    stride_bytes = elem_step * mybir.dt.size(in_ap.dtype)
    stride_256 = stride_bytes // 256
    assert stride_256 * 256 == stride_bytes and stride_256 < 256
    _in_ap = gp.lower_ap_dma(in_ap, for_custom_bir_dma=True)
    _idxs_ap = gp.lower_ap(idxs_ap)
    _out_ap = gp.lower_ap(out_ap)
    return gp.add_instruction(
        mybir.InstDMAGatherAnt(
            name=gp.bass.get_next_instruction_name(),
            ins=[*_in_ap, _idxs_ap, gp.lower_val_access(gp.to_reg(num_idxs))],
            outs=[_out_ap],
            transpose=False,
            num_idxs=num_idxs,
            elem_size=elem_size,
            stride_bytes_256=stride_256,
            gen_mode=0,
            single_packet=False,
            queue_num=0,
            sbuf_tokens_per_rank=0,
            sbuf_free_dim_per_rank=0,
            sbuf_free_dim_pad_per_rank=0,
            sbuf_byte_offset=0,
        ))


def _sched(total, tiny_tail=False):
    """Chunk schedule: halved first chunks (shrinks the phase-boundary
    pipeline bubble) and, for L3, a tiny last chunk (shrinks the final
    compute tail after the last gather)."""
    sizes = [30, 30] if total > MCHUNK else []
    left = total - sum(sizes)
    while left > MCHUNK:
        sizes.append(MCHUNK)
        left -= MCHUNK
    if tiny_tail and left > 10:
        sizes += [left - 10, 10]
    elif left:
        sizes.append(left)
    out, s0 = [], 0
    for n in sizes:
        out.append((s0, n))
        s0 += n
    return out


def _gathers(nslots):
    """Split a chunk of nslots*128 idxs into per-instruction counts."""
    n = nslots * 128
    out = []
    while n > 0:
        g = min(GPN, n)
        out.append(g)
        n -= g
    return out


S1 = 512        # layer-1 grid slots: 8 affine x-columns + spill tail
NPOS1 = S1 * 128


def build_nc(bounds=None, l0meta=None, gslot=None, dims=None):
    """l0meta: (cols, tail_ns) — cols = [(s0, ns)] for the 8 affine
    x-columns of layer 1 (a read via broadcast from xs, no a-gather),
    tail_ns = slots of the spill chunk (both lists gathered)."""
    if "nc" in _NC_CACHE:
        return _NC_CACHE["nc"]
    nc = bacc.Bacc("TRN2", target_bir_lowering=False, debug=False,
                   enable_asserts=False, num_devices=NCORES)

    # dims: (S1U, S2) — used L1 grid slots (affine+tail) and pruned L2
    # slot count (dead neurons dropped; see kernel()).
    S1U, S2 = dims
    SL = [S1U, S2, NSLOT]          # compute-slots per layer
    NQL = [(S1U + 7) // 8, (S2 + 7) // 8, (NSLOT + 7) // 8]
    NQ1 = S1 // 8                  # max coef-quad count (tile sizing)
    if gslot is not None:
        gfirst = {g: min(np.nonzero(gslot == g)[0]) for g in range(K)}
        glast = {g: max(np.nonzero(gslot == g)[0]) for g in range(K)}

    xT = nc.dram_tensor("xT", [IN_DIM, E], BF16, kind="ExternalInput")
    xs = nc.dram_tensor("xs", [128, 8, 32], BF16, kind="ExternalInput")
    # wf2[(k,m), q, K] = w[neuron(p=K, s=8q+m), k]  (PE-coef layout)
    FP8 = mybir.dt.float8e4
    wf = [nc.dram_tensor(f"wf{l}", [128, NQL[l], 128], FP8,
                         kind="ExternalInput") for l in range(3)]
    ia0 = nc.dram_tensor("ia0", [128, 256], I16, kind="ExternalInput")
    ib0 = nc.dram_tensor("ib0", [128, S1U * 8], I16, kind="ExternalInput")
    # merged a+b idx stream per chunk for layers 2-3 (one gather/chunk)
    iab = [None] + [
        nc.dram_tensor(f"iab{l}", [128, 2 * SL[l] * 8], I16,
                       kind="ExternalInput") for l in (1, 2)]
    # ghat[(k,m), 5j+mm] = G[k, j] * (m == mm); j=4 row is ones (softmax denom)
    ghat = nc.dram_tensor("ghat", [128, 40], BF16, kind="ExternalInput")
    h_dram = [nc.dram_tensor("h0", [NPOS1, E], BF16, kind="Internal"),
              nc.dram_tensor("h1", [S2 * 128, E], BF16, kind="Internal")]
    out_dram = nc.dram_tensor("out", [K, BC], F32, kind="ExternalOutput")

    with tile.TileContext(nc) as tc:
        with (
            tc.tile_pool(name="persist", bufs=1) as persist,
            tc.tile_pool(name="idxp", bufs=2) as idxp,
            tc.tile_pool(name="coef", bufs=1) as coefp,
            tc.tile_pool(name="cjdp", bufs=1) as cjdp,
            tc.tile_pool(name="gath", bufs=5) as gath,
            tc.tile_pool(name="temps", bufs=4) as temps,
            tc.tile_pool(name="psum", bufs=1, space="PSUM") as psump,
        ):
            nc.gpsimd.load_library(mlp)

            # persistent loads (idx tensors go first in the layer loop;
            # these small loads follow so the first gather starts ASAP)
            ghat_sb = persist.tile([128, 40], BF16, tag="ghat")
            ones_sb = persist.tile([128, BC], BF16, tag="ones")
            nc.vector.memset(ones_sb[:], 1.0)

            # per-group accumulators in SEPARATE 2KB PSUM banks: a matmul
            # with start=True marks its whole 2KB zero-region pending-zero,
            # so interleaved group starts sharing a bank wipe each other.
            # Groups 0-4 -> partition row 0 banks 0-4; groups 5-9 -> row 32
            # (PE out base partition must be 0/32/64).
            psum_out = psump.tile([64, 5, 512], F32, tag="acc")

            def psum_g(g):
                return (psum_out[0:1, g, 0:BC] if g < 5
                        else psum_out[32:33, g - 5, 0:BC])

            xs_sb = persist.tile([128, 8, 32], BF16, tag="xs")

            for l in range(3):
                NQl = NQL[l]
                Sl = SL[l]
                if l == 0:
                    ia_sb = idxp.tile([128, 256], I16, tag="iat", name="ia_sb")
                    ib_sb = idxp.tile([128, S1U * 8], I16, tag="ib0t",
                                      name="ib_sb")
                    nc.sync.dma_start(ia_sb[:], ia0[:])
                    nc.sync.dma_start(ib_sb[:], ib0[:])
                    nc.sync.dma_start(xs_sb[:], xs[:])
                    nc.sync.dma_start(ghat_sb[:], ghat[:])
                else:
                    iab_sb = idxp.tile([128, 2 * SL[l] * 8], I16, tag="iab",
                                       name="iab_sb")
                    lc0 = 0
                    for _, lns in _sched(SL[l], l == 2):
                        lnc = 2 * lns * 8
                        nc.sync.dma_start(iab_sb[:, lc0:lc0 + lnc],
                                          iab[l][:, lc0:lc0 + lnc])
                        lc0 += lnc

                # ---- coefficient prep: coef = softmax(w) @ GATE_COEF via PE ----
                # craw[j, n] for neurons n=(K, s=8q+m): matmul q contracts over
                # the (gate k, m) partition dim: out[K, j*8+mm] = sum_p
                # e_fold[p, K] * ghat[p, j*8+mm].
                wf_sb = coefp.tile([128, NQ1, 128], mybir.dt.float8e4,
                                   tag="wf", name="wf_sb")
                nc.sync.dma_start(wf_sb[:, :NQl, :], wf[l][:])
                e_sb = coefp.tile([128, NQ1, 128], BF16, tag="e", name="e_sb")
                nc.scalar.activation(e_sb[:, :NQl, :], wf_sb[:, :NQl, :],
                                     mybir.ActivationFunctionType.Exp)
                c_sb = coefp.tile([128, NQ1, 40], BF16, tag="csb", name="c_sb")
                QG = 12  # matmuls per PSUM bank group
                q0 = 0
                while q0 < NQl:
                    nq = min(QG, NQl - q0)
                    cps = psump.tile([128, QG, 40], F32, tag="cps", name="cps")
                    for qi in range(nq):
                        nc.tensor.matmul(cps[:, qi, :],
                                         lhsT=e_sb[:, q0 + qi, :],
                                         rhs=ghat_sb[:],
                                         start=True, stop=True)
                    nc.scalar.mul(c_sb[:, q0:q0 + nq, :], cps[:, :nq, :], 1.0)
                    q0 += nq
                rinv = coefp.tile([128, NQ1, 8], F32, tag="rinv", name="rinv")
                nc.vector.reciprocal(out=rinv[:, :NQl, :], in_=c_sb[:, :NQl, 32:40])
                # cjd[j][p, s, 0:2] = coef_j[p, s] twice (stride-1 pair so the
                # gate ops' broadcast operand keeps the DVE 2x perf mode)
                cjd = [cjdp.tile([128, NQ1 * 8, 2], BF16, tag=f"c{j}",
                                  name=f"cjd{j}") for j in range(4)]
                for j in range(4):
                    dst = (cjd[j][:, :NQl * 8, :]
                           .rearrange("p (q m) t -> p q m t", m=8))
                    cj_s = (c_sb[:, :NQl, j * 8:(j + 1) * 8].unsqueeze(-1)
                            .to_broadcast([128, NQl, 8, 2]))
                    ri_s = (rinv[:, :NQl, :].unsqueeze(-1)
                            .to_broadcast([128, NQl, 8, 2]))
                    nc.vector.tensor_mul(dst, cj_s, ri_s)

                # ---- gather + gate over chunks ----
                # L1: per-column b-gathers (a affine) + dual-gather tail.
                # L2/L3: ONE merged a+b gather per chunk (idx stream =
                # [a-idxs, b-idxs]; shared signed-int16 rebase; halves the
                # SWDGE fixed overhead so desc-gen keeps up with the DMA).
                if l == 0:
                    cols, tail_ns = l0meta
                    s_aff = cols[-1][0] + cols[-1][1]
                    chunks = [(s0, ns, i) for i, (s0, ns) in enumerate(cols)]
                    if tail_ns:
                        chunks.append((s_aff, tail_ns, None))
                else:
                    chunks = [(s0, ns, None)
                              for s0, ns in _sched(Sl, l == 2)]

                gi = 0
                acol = 0  # a-idx cols consumed (layer-0 tail only)
                mcol = 0  # merged idx cols consumed (layers 2-3)
                for s0, ns, xcol in chunks:
                    if l == 0:
                        a_t = gath.tile([128, CHUNK_SLOTS, 32], BF16, tag="a")
                        b_t = gath.tile([128, CHUNK_SLOTS, 32], BF16, tag="b")
                        col = s0 * 8  # b-idx cols consumed (128/16 per slot)
                        slot = 0
                        for n in _gathers(ns):
                            ncols = n // 16
                            nslots_g = n // 128
                            if xcol is None:
                                dma_gather_small(
                                    nc.gpsimd,
                                    a_t[:, slot:slot + nslots_g, :],
                                    xT[:, :32],
                                    ia_sb[:, acol:acol + ncols], n, 32, E)
                                acol += ncols
                            dma_gather_small(
                                nc.gpsimd, b_t[:, slot:slot + nslots_g, :],
                                xT[:, :32],
                                ib_sb[:, col:col + ncols], n, 32, E)
                            col += ncols
                            slot += nslots_g
                    else:
                        assert bounds is not None
                        ba, bda = bounds[l - 1]
                        ab_t = gath.tile([128, 2 * MCHUNK, 32], BF16,
                                         tag="ab")
                        a_t = ab_t[:, :ns, :]
                        b_t = ab_t[:, ns:2 * ns, :]
                        n = 2 * ns * 128
                        src = h_dram[l - 1][ba[gi]:bda[gi], :32]
                        gi += 1
                        dma_gather_small(
                            nc.gpsimd, ab_t[:, :2 * ns, :], src,
                            iab_sb[:, mcol:mcol + n // 16], n, 32, E)
                        mcol += n // 16

                    # 4-dim views with stride-1 inner pairs keep DVE 2x mode
                    if l > 0:
                        av = a_t.rearrange("p c (g t) -> p c g t", t=2)
                        bv = b_t.rearrange("p c (g t) -> p c g t", t=2)
                    elif xcol is None:
                        av = a_t[:, :ns, :].rearrange("p c (g t) -> p c g t", t=2)
                        bv = b_t[:, :ns, :].rearrange("p c (g t) -> p c g t", t=2)
                    else:
                        av = (xs_sb[:, xcol, :].rearrange("p (g t) -> p g t", t=2)
                              .unsqueeze(1).to_broadcast([128, ns, 16, 2]))
                        bv = b_t[:, :ns, :].rearrange("p c (g t) -> p c g t", t=2)

                    def cbc(j):
                        return (cjd[j][:, s0:s0 + ns, :].unsqueeze(2)
                                .to_broadcast([128, ns, 16, 2]))

                    m1 = temps.tile([128, CHUNK_SLOTS, 32], BF16, tag="m1")
                    m1v = m1[:, :ns, :].rearrange("p c (g t) -> p c g t", t=2)
                    if l < 2:
                        # gate = (c1 + c3*b)*a + (c0 + c2*b): 6 DVE ops
                        m2 = temps.tile([128, CHUNK_SLOTS, 32], BF16, tag="m2")
                        m2v = m2[:, :ns, :].rearrange("p c (g t) -> p c g t",
                                                      t=2)
                        nc.vector.tensor_mul(m1v, bv, cbc(3))
                        nc.vector.tensor_add(m1v, m1v, cbc(1))
                        nc.vector.tensor_mul(m1v, m1v, av)
                        nc.vector.tensor_mul(m2v, bv, cbc(2))
                        nc.vector.tensor_add(m2v, m2v, cbc(0))
                        nc.vector.tensor_add(m1v, m1v, m2v)
                        # write rows (s0+c)*128+p of h_dram[l] (64B @ 256B stride)
                        hap = h_dram[l].ap()
                        dst = hap[s0 * 128: s0 * 128 + ns * 128, :32]
                        dst = dst.rearrange("(c p) e -> p c e", p=128)
                        nc.sync.dma_start(dst, m1[:, :ns, :])
                    else:
                        # L3: y = c0 + c1 a + c2 b + c3 ab summed per group.
                        # Slots are group-pure (host bucketing): feed a, b,
                        # ab straight to PE with coef-column lhsT — only
                        # 1 DVE op (ab) per chunk.
                        nc.vector.tensor_mul(m1v, av, bv)
                        for c in range(ns):
                            cs = s0 + c
                            g = int(gslot[cs])
                            for fj, rhs in ((0, ones_sb[:]),
                                            (1, ab_t[:, c, :]),
                                            (2, ab_t[:, ns + c, :]),
                                            (3, m1[:, c, :])):
                                nc.tensor.matmul(
                                    psum_g(g),
                                    lhsT=cjd[fj][:, cs, 0:1],
                                    rhs=rhs,
                                    start=(cs == gfirst[g] and fj == 0),
                                    stop=(cs == glast[g] and fj == 3),
                                )

            out_sb = persist.tile([64, 5, BC], F32, tag="outsb")
            pv = psum_out[:].rearrange("p g (u b) -> p g u b", b=BC)[:, :, 0, :]
            nc.scalar.mul(out_sb[0:1, :, :], pv[0:1, :, :], 1.0 / TAU)
            nc.scalar.mul(out_sb[32:33, :, :], pv[32:33, :, :], 1.0 / TAU)
            dst = out_dram.ap().rearrange("(r k) b -> r k b", r=2)
            nc.sync.dma_start(dst, out_sb[0:64:32, :, :])

    nc.compile()
    _NC_CACHE["nc"] = nc
    return nc


def _wrap(idx):
    """Flat idx list [n] -> [128, n/16] int16 wrapped per 16 partitions,
    replicated to the 8 gpsimd cores."""
    n = idx.shape[0]
    arr = np.empty((128, n // 16), dtype=np.int16)
    blk = idx.reshape(n // 16, 16).T.astype(np.int16)
    for g in range(8):
        arr[g * 16:(g + 1) * 16, :] = blk
    return arr


def kernel(x, w1, w2, w3, idx_a1, idx_b1, idx_a2, idx_b2, idx_a3, idx_b3):
    x = np.asarray(x, dtype=np.float32)
    ws = [np.asarray(w, dtype=np.float32) for w in (w1, w2, w3)]
    ias = [np.asarray(i).astype(np.int64) for i in (idx_a1, idx_a2, idx_a3)]
    ibs = [np.asarray(i).astype(np.int64) for i in (idx_b1, idx_b2, idx_b3)]

    # ---- host-side index translation / layout prep (shared across cores) ----
    # Dead-neuron pruning: L2 neurons never gathered by L3 and L1 neurons
    # never gathered by the (pruned) L2 need no gather/gate/write at all.
    alive2 = np.zeros(W, dtype=bool)
    alive2[ias[2]] = True
    alive2[ibs[2]] = True
    l2_ids = np.nonzero(alive2)[0]
    alive1 = np.zeros(W, dtype=bool)
    alive1[ias[1][l2_ids]] = True
    alive1[ibs[1][l2_ids]] = True
    l1_ids = np.nonzero(alive1)[0]

    # Layer 1: assign x value of count-rank r to cell (p=r%128, col=r//128);
    # consumers sit in their a-source's cell (up to the column cap), so the
    # a-operand is an affine broadcast from xs and needs NO gather. Excess
    # consumers spill to a tail chunk where both lists are gathered.
    T_CAP = 64
    a1f = ias[0][l1_ids]
    counts = np.bincount(a1f, minlength=IN_DIM)
    order_v = np.argsort(-counts, kind="stable")
    Mcol = np.minimum(counts[order_v].reshape(8, 128).max(axis=1),
                      T_CAP).astype(np.int64)
    offs = np.concatenate([[0], np.cumsum(Mcol)])
    S_AFF = int(Mcol.sum())
    cols = [(int(offs[i]), int(Mcol[i])) for i in range(8)]
    ordc = l1_ids[np.argsort(a1f, kind="stable")]
    starts = np.concatenate([[0], np.cumsum(counts)])
    perm0 = np.full(NPOS1, -1, dtype=np.int64)
    tail_cons = []
    for r in range(IN_DIM):
        v = int(order_v[r])
        p, i = r % 128, r // 128
        cons = ordc[starts[v]:starts[v + 1]]
        k = min(len(cons), int(Mcol[i]))
        perm0[(offs[i] + np.arange(k)) * 128 + p] = cons[:k]
        tail_cons.extend(cons[k:].tolist())
    TAIL = (len(tail_cons) + 127) // 128
    S1U = S_AFF + TAIL
    assert S1U <= S1 and TAIL * 128 <= 4096
    perm0[S_AFF * 128 + np.arange(len(tail_cons))] = tail_cons
    mask0 = perm0 >= 0
    b1 = np.zeros(S1U * 128, dtype=np.int64)
    mu = mask0[:S1U * 128]
    b1[mu] = ibs[0][perm0[:S1U * 128][mu]]
    a_tail = np.zeros(TAIL * 128, dtype=np.int64)
    tmask = mask0[S_AFF * 128:S1U * 128]
    a_tail[tmask] = ias[0][perm0[S_AFF * 128:S1U * 128][tmask]]
    a_tail_pad = np.zeros(256 * 16, dtype=np.int64)
    a_tail_pad[:TAIL * 128] = a_tail
    K2 = len(l2_ids)
    S2 = (K2 + 127) // 128
    K2p = S2 * 128

    perms = [perm0]
    lists = [None]
    bounds = []
    for l in (1, 2):
        pm = perms[l - 1]
        inv_prev = np.full(W, -1, dtype=np.int64)
        msk = pm >= 0
        inv_prev[pm[msk]] = np.nonzero(msk)[0]
        if l == 1:
            # pruned consumer set, padded to a slot boundary with dummies
            ca = np.full(K2p, l1_ids[0], dtype=np.int64)
            cb = np.full(K2p, l1_ids[0], dtype=np.int64)
            ca[:K2] = ias[l][l2_ids]
            cb[:K2] = ibs[l][l2_ids]
            cons_ids = np.full(K2p, -1, dtype=np.int64)
            cons_ids[:K2] = l2_ids
        else:
            ca, cb = ias[l], ibs[l]
            cons_ids = np.arange(W)
        ra = inv_prev[ca]
        rb = inv_prev[cb]
        assert (ra >= 0).all() and (rb >= 0).all()
        # sort consumers by max source row: early gather instructions then
        # only touch early h rows, so (with per-instruction src-AP windows)
        # they can start before the previous layer finishes writing h.
        # L3 additionally needs group-pure SLOTS (GroupSum via per-slot
        # coef-column matmuls): walk consumers in global max-order and
        # bucket per group, emitting a slot whenever a bucket fills 128 —
        # slot order stays ~max-sorted so gather windows stay narrow.
        order = np.argsort(np.maximum(ra, rb), kind="stable")
        if l == 2:
            grp = cons_ids[order] // (W // K)
            buckets = [[] for _ in range(K)]
            out_pos = []
            gslot = []
            for j, g in zip(order, grp):
                b = buckets[g]
                b.append(j)
                if len(b) == 128:
                    out_pos.extend(b)
                    gslot.append(int(g))
                    buckets[g] = []
            assert not any(buckets) and len(gslot) == NSLOT
            order = np.array(out_pos, dtype=np.int64)
            group_of_slot = np.array(gslot, dtype=np.int64)
            ra2, rb2 = ra[order].copy(), rb[order].copy()
        else:
            ra2, rb2 = ra[order].copy(), rb[order].copy()
        # per-chunk signed-int16 rebase shared by the merged a+b stream:
        # base = max(0, hi-32767); idxs below base go NEGATIVE and the
        # SWDGE address math reaches backward (span <= 65535 always fits).
        # Only TRAILING negatives are trimmed by the ucode, so ensure the
        # merged stream's final idx (b-part tail) is >= 0 via a consumer
        # swap inside the LAST SLOT of the chunk (slots are group-pure).
        Sl = S2 if l == 1 else NSLOT
        binfo = ([], [])  # base, bound per merged chunk
        pos = 0
        for s0, ns in _sched(Sl, l == 2):
            sl = slice(pos, pos + ns * 128)
            hi = int(max(ra2[sl].max(), rb2[sl].max()))
            base = max(0, hi - 32767)
            ra2[sl] -= base
            rb2[sl] -= base
            binfo[0].append(base)
            binfo[1].append(hi + 1)
            last = pos + ns * 128 - 1
            if rb2[last] < 0:
                lo = last - 127  # same slot -> same group
                okc = np.nonzero(rb2[lo:last] >= 0)[0]
                j = lo + int(okc[-1])
                for arr in (ra2, rb2, order):
                    arr[last], arr[j] = arr[j], arr[last]
            pos += ns * 128
        perms.append(cons_ids[order])
        lists.append((ra2, rb2))
        bounds.append(binfo)

    nc = build_nc(bounds, l0meta=(cols, TAIL), gslot=group_of_slot,
                  dims=(S1U, S2))

    NQ3 = (NSLOT + 7) // 8
    NQ2 = (S2 + 7) // 8
    NQ1 = S1 // 8

    def _wf_fold(wp, S, NQl):
        # wf2[k*8+m, q, K] = wp[(8q+m)*128 + K, k]
        wf2 = np.zeros((128, NQl, 128), dtype=np.float32)
        for m in range(8):
            s_ids = 8 * np.arange(NQl) + m
            valid = s_ids < S
            n = s_ids[valid][:, None] * 128 + np.arange(128)[None, :]
            vals = wp[n, :].transpose(2, 0, 1)    # [16, nq_v, 128]
            tmp = np.zeros((16, NQl, 128), dtype=np.float32)
            tmp[:, valid, :] = vals
            wf2[np.arange(16) * 8 + m] = tmp
        return wf2.astype(ml_dtypes.float8_e4m3)

    shared = {}
    shared["ia0"] = _wrap(a_tail_pad)
    shared["ib0"] = _wrap(b1)
    wp0 = np.zeros((NPOS1, 16), dtype=np.float32)
    wp0[mask0] = ws[0][perm0[mask0]]
    shared["wf0"] = _wf_fold(wp0, S1U, (S1U + 7) // 8)
    for l, Sl, NQl in ((1, S2, NQ2), (2, NSLOT, NQ3)):
        a, b = lists[l]
        stream, pos = [], 0
        for s0, ns in _sched(Sl, l == 2):
            stream.append(a[pos:pos + ns * 128])
            stream.append(b[pos:pos + ns * 128])
            pos += ns * 128
        shared[f"iab{l}"] = _wrap(np.concatenate(stream))
        wp = np.zeros((Sl * 128, 16), dtype=np.float32)
        pmm = perms[l] >= 0
        wp[pmm] = ws[l][perms[l][pmm]]
        shared[f"wf{l}"] = _wf_fold(wp, Sl, NQl)

    # L3 slots are group-pure by construction (bucketed); verify.
    group = perms[2] // (W // K)
    assert (group == np.repeat(group_of_slot, 128)).all()

    G5 = np.zeros((16, 5), dtype=np.float32)
    G5[:, :4] = GATE_COEF
    G5[:, 4] = 1.0
    ghat = np.zeros((128, 40), dtype=np.float32)
    for k in range(16):
        for m in range(8):
            ghat[k * 8 + m, np.arange(5) * 8 + m] = G5[k]
    shared["ghat"] = ghat.astype(ml_dtypes.bfloat16)

    in_maps = []
    vids = order_v.reshape(8, 128)
    for c in range(NCORES):
        xc = x[c * BC:(c + 1) * BC]               # [32, 1024]
        xt = np.zeros((IN_DIM, E), dtype=ml_dtypes.bfloat16)
        xt[:, :BC] = xc.T.astype(ml_dtypes.bfloat16)
        m = dict(shared)
        m["xT"] = xt
        # xs[p, i, :] = x batch vector of the value at cell (p, i)
        m["xs"] = np.ascontiguousarray(
            xc[:, vids].transpose(2, 1, 0)).astype(ml_dtypes.bfloat16)
        in_maps.append(m)

    res = run_bass_kernel_spmd(nc, in_maps, core_ids=list(range(NCORES)))

    out = np.empty((BATCH, K), dtype=np.float32)
    for c in range(NCORES):
        out[c * BC:(c + 1) * BC] = res.results[c]["out"].T
    return out

